# revision 33
# baseline (speedup 1.0000x reference)
"""Class-align loss (segment_reduce) Trainium2 kernel.

Full inputs: f_source [4,256,128,128] f32, f_convert [4,256,128,128] f32,
seg [4,128,128] int32 (values in [0,19)). Output: scalar f32 triplet loss.

Strategy (data-parallel over batch*h-half, 8 shards):
  - Each core processes a [256, 8192] shard of each feature tensor
    (1 batch x 64 h-rows x 128 w). Staging DMAs cast fp32 -> bf16 in
    flight (SWDGE); accumulation stays fp32 in PSUM.
  - Pixels are processed in batches of 4 groups x 128 pixels: PE
    transposes eight [128c,128p] bf16 blocks into one full-bank PSUM
    tile, one DVE copy moves it to SBUF; ACT computes per-pixel
    sum-of-squares (Square + accum) per group; one batched sqrt (ACT)
    + reciprocal (DVE) per batch gives r = 1/||x||.
  - Per-pixel normalization is folded into the one-hot class weights
    w[p,k] = (seg[p]==k) * r[p]; PE accumulates transposed class sums
    accT[c_half,k] += xT_half.T @ w (data is the 128-col stationary
    operand -> fast weight load; w is the 19-col moving operand).
  - Each core writes its partial [2,2,128,19] sums; the host sums the
    8 partials and computes the tiny (19-class) normalize +
    triplet-loss epilogue in float64.

The walrus build used here encodes at most ONE sync wait per
instruction. Everything below is arranged so no instruction ever needs
two: staging tiles are dedicated (wait-free DMAs), absorber transposes
take the staging-DMA waits on PE, sync=False ordering edges keep the PE
stream near program order so PSUM-slot WAR waits are subsumed by the
vector clock, and the kernel-tail drain is split across sequencer NOPs.
"""

import sys

import numpy as np

if "/opt/trn_rl_repo" not in sys.path:
    sys.path.insert(0, "/opt/trn_rl_repo")

import concourse.bass as bass
import concourse.mybir as mybir
import concourse.tile as tile
from concourse.bass_utils import run_bass_kernel_spmd
from concourse.tile import add_dep_helper
from concourse.vector_clock import ScopedClock


def _split_drain_and_barrier(self, tick_clock, wait_clock):
    """Tile's kernel-tail drain carries one wait per semaphore the kernel
    ever used; split the excess onto dedicated sequencer NOPs (the 1-wait
    walrus encoding limit)."""
    nc = self.nc
    drain_inst = nc.sync.drain()
    wait_clock.add_sem_waits(
        drain_inst.ins, ScopedClock({None: tick_clock.global_clock})
    )
    si = drain_inst.ins.sync_info
    if si is not None and len(si.on_wait) > 1:
        waits = list(si.on_wait)
        upds = list(si.on_update)
        drain_inst.ins.sync_info = mybir.SyncInfo(
            on_wait=waits[:1], on_update=upds)
        for k in range(1, len(waits)):
            nop = nc.sync.nop(nofuse=True, hint=f"drain_wait_{k}")
            nop.ins.sync_info = mybir.SyncInfo(
                on_wait=[waits[k]], on_update=[])
    nc.all_engine_barrier()
    assert self.sems is not None
    popped = nc._tile_sem_poison_stack.pop()
    assert popped is self._sem_poison
    nc.clear_and_free_semaphores(list(self.sems.allocated().values()))
    nc.all_engine_barrier()


tile.TileContext._drain_and_barrier = _split_drain_and_barrier

# Problem constants (hardcoded; kernel.py must be self-contained).
B, C, H, W = 4, 256, 128, 128
N_CLASS = 19
N_CORES = 8
EPS_NORM = 1e-12
EPS_TRIP = 1e-6
MARGIN = 0.2

P = 128                      # SBUF partitions / pixel-group size
NPIX = B * H * W // N_CORES  # 8192 pixels per core
NG = NPIX // P               # 64 pixel groups per core
GPB = 4                      # pixel groups per batch (one PSUM bank)
NB = NG // GPB               # 16 batches per tensor
CHUNK_PIX = 4096             # pixels staged per DMA
NCHUNK = NPIX // CHUNK_PIX   # 4
BPC = CHUNK_PIX // (P * GPB) # 4 batches per chunk

_NC_CACHE = {}


def build_nc():
    f32 = mybir.dt.float32
    bf16 = mybir.dt.bfloat16
    i32 = mybir.dt.int32
    nc = bass.Bass()

    fs_dram = nc.declare_dram_parameter("f_source", [C, NPIX], f32, isOutput=False)
    aux_dram = nc.declare_dram_parameter("aux", [P, P + N_CLASS], f32,
                                         isOutput=False)
    fc_dram = nc.declare_dram_parameter("f_convert", [C, NPIX], f32, isOutput=False)
    seg_dram = nc.declare_dram_parameter("seg", [NPIX], i32, isOutput=False)
    out_dram = nc.declare_dram_parameter("out", [2, 2, P, N_CLASS], f32,
                                         isOutput=True)

    with tile.TileContext(nc) as tc:
        with (
            tc.tile_pool(name="const", bufs=1) as const_pool,
            tc.tile_pool(name="stage", bufs=1) as stage_pool,
            tc.tile_pool(name="work", bufs=4) as work_pool,
            tc.tile_pool(name="wpool", bufs=16) as w_pool,
            tc.tile_pool(name="psum_t", bufs=3, space="PSUM") as psum_t_pool,
            tc.tile_pool(name="psum_abs", bufs=1, space="PSUM") as psum_abs_pool,
            tc.tile_pool(name="psum_acc", bufs=1, space="PSUM") as psum_acc_pool,
        ):
            # identity + iota row arrive via DMA (the "aux" input): building
            # them with gpsimd would add the Pool semaphore to every
            # wait-budget discussion.
            aux_sb = const_pool.tile([P, P + N_CLASS], f32, tag="aux")
            nc.gpsimd.dma_start(out=aux_sb[:], in_=aux_dram[:])
            iota19 = aux_sb[:, P:P + N_CLASS]
            ident_bf = const_pool.tile([P, P], bf16, tag="ident_bf")
            nc.vector.tensor_copy(ident_bf[:], aux_sb[:, 0:P])
            identity = ident_bf[:]

            # seg wanted as [pixel-within-group (partition), group (free)].
            # A strided gather DMA would cost 8192 descriptors (~48us of Q7
            # descriptor generation), so load contiguously and PE-transpose.
            seg_i = const_pool.tile([NG, P], i32, tag="seg_i")
            nc.gpsimd.dma_start(
                out=seg_i[:], in_=seg_dram[:].rearrange("(g p) -> g p", p=P))
            seg_f = const_pool.tile([NG, P], f32, tag="seg_f")
            nc.vector.tensor_copy(seg_f[:], seg_i[:])
            ident_sm = const_pool.tile([NG, NG], f32, tag="ident_sm")
            nc.vector.tensor_copy(ident_sm[:], aux_sb[:NG, :NG])
            seg_ps = psum_t_pool.tile([P, NG], f32, tag="pt", name="seg_ps",
                                      padded_shape=[P, 512])
            nc.tensor.transpose(seg_ps[:], seg_f[:], ident_sm[:])
            seg_sb = const_pool.tile([P, NG], f32, tag="seg")
            nc.vector.tensor_copy(seg_sb[:], seg_ps[:])

            # Dummy DVE read of iota19: pre-syncs DVE against the aux DMA so
            # the first w-generation op carries a single wait.
            iota_warm = const_pool.tile([P, N_CLASS], f32, tag="iota_warm")
            nc.vector.tensor_copy(iota_warm[:], iota19)

            # Warm-up transpose: pre-syncs PE against ident_bf (DVE).
            warm = psum_t_pool.tile([P, P], bf16, tag="pt", name="warm",
                                    padded_shape=[P, 1024])
            nc.tensor.transpose(warm[:, 0:P], identity, identity)

            # Transposed fp32 accumulators: accT[tensor][half] = [c_half, k].
            accs = {
                (t, h): psum_acc_pool.tile([P, N_CLASS], f32,
                                           tag=f"acc_{t}{h}", name=f"acc_{t}{h}")
                for t in ("s", "c") for h in (0, 1)
            }
            drams = {"s": fs_dram, "c": fc_dram}

            # Dedicated bank for the DMA-wait absorber transposes (never
            # read; lo/hi slices are byte-disjoint).
            absorb = psum_abs_pool.tile([P, 8 * P], bf16, tag="absorb",
                                        name="absorb", padded_shape=[P, 1024])

            mm_all = []

            def order_after_mm(inst, back=24):
                if len(mm_all) >= back:
                    add_dep_helper(inst.ins, mm_all[-back].ins, sync=False,
                                   reason="keep PE stream near program order")

            for ci in range(NCHUNK):
                for t in ("s", "c"):
                    # Dedicated staging tiles per (chunk, tensor, half): the
                    # DMAs carry zero waits. SWDGE casts fp32->bf16 in flight.
                    lo = stage_pool.tile([P, CHUNK_PIX], bf16,
                                         tag=f"{t}_lo_{ci}", name=f"{t}_lo_{ci}")
                    hi = stage_pool.tile([P, CHUNK_PIX], bf16,
                                         tag=f"{t}_hi_{ci}", name=f"{t}_hi_{ci}")
                    pix0 = ci * CHUNK_PIX
                    nc.gpsimd.dma_start(
                        out=lo[:], in_=drams[t][0:P, pix0:pix0 + CHUNK_PIX])
                    nc.gpsimd.dma_start(
                        out=hi[:], in_=drams[t][P:C, pix0:pix0 + CHUNK_PIX])
                    ab1 = nc.tensor.transpose(absorb[:, 0:P], lo[:, 0:P],
                                              identity)
                    ab2 = nc.tensor.transpose(absorb[:, P:2 * P], hi[:, 0:P],
                                              identity)
                    order_after_mm(ab1)
                    order_after_mm(ab2)
                    for bi in range(BPC):
                        batch = ci * BPC + bi            # global batch index
                        # One full PSUM bank holds 4 groups x [128p, 256c].
                        psumT = psum_t_pool.tile([P, GPB * C], bf16, tag="pt",
                                                 padded_shape=[P, 1024])
                        tps = []
                        for g in range(GPB):
                            px = (bi * GPB + g) * P
                            t1 = nc.tensor.transpose(
                                psumT[:, g * C:g * C + P],
                                lo[:, px:px + P], identity)
                            t2 = nc.tensor.transpose(
                                psumT[:, g * C + P:(g + 1) * C],
                                hi[:, px:px + P], identity)
                            tps.extend((t1, t2))
                        for tp in tps:
                            order_after_mm(tp)
                        xT = work_pool.tile([P, GPB * C], bf16, tag="xT",
                                              bufs=6)
                        nc.vector.tensor_copy(xT[:], psumT[:])

                        ss = work_pool.tile([P, GPB], f32, tag="ss", bufs=32)
                        sq4 = work_pool.tile([P, GPB * C], bf16, tag="sq4",
                                             bufs=32)
                        nc.scalar.activation(
                            sq4[:], xT[:],
                            mybir.ActivationFunctionType.Square)
                        nc.vector.tensor_reduce(
                            out=ss[:],
                            in_=sq4[:].rearrange("p (g c) -> p g c", g=GPB),
                            axis=mybir.AxisListType.X,
                            op=mybir.AluOpType.add)
                        nrm = work_pool.tile([P, GPB], f32, tag="nrm", bufs=32)
                        nc.scalar.sqrt(nrm[:], ss[:])
                        r = work_pool.tile([P, GPB], f32, tag="r", bufs=32)
                        nc.vector.reciprocal(r[:], nrm[:])

                        for g in range(GPB):
                            G = batch * GPB + g
                            w = w_pool.tile([P, N_CLASS], bf16, tag="w")
                            nc.vector.tensor_scalar(
                                out=w[:], in0=iota19,
                                scalar1=seg_sb[:, G:G + 1],
                                scalar2=r[:, g:g + 1],
                                op0=mybir.AluOpType.is_equal,
                                op1=mybir.AluOpType.mult)
                            for h in (0, 1):
                                mm = nc.tensor.matmul(
                                    accs[(t, h)][:],
                                    lhsT=xT[:, g * C + h * P:g * C + (h + 1) * P],
                                    rhs=w[:],
                                    start=(G == 0), stop=(G == NG - 1))
                                mm_all.append(mm)

            out_sb = work_pool.tile([P, 4 * N_CLASS], f32, tag="out_sb")
            for j, (t, h) in enumerate(((("s", 0)), ("s", 1), ("c", 0), ("c", 1))):
                nc.vector.tensor_copy(
                    out_sb[:, j * N_CLASS:(j + 1) * N_CLASS], accs[(t, h)][:])
            for j, (ti, h) in enumerate(((0, 0), (0, 1), (1, 0), (1, 1))):
                # HWDGE lanes are otherwise unused -> each DMA carries only
                # its DVE wait.
                nc.sync.dma_start(
                    out=out_dram[ti, h],
                    in_=out_sb[:, j * N_CLASS:(j + 1) * N_CLASS])

    return nc


def aux_array():
    ident = np.eye(P, dtype=np.float32)
    iota = np.tile(np.arange(N_CLASS, dtype=np.float32), (P, 1))
    return np.ascontiguousarray(np.concatenate([ident, iota], axis=1))


def shard_inputs(f_source, f_convert, seg):
    """Split by (batch, h-half) into 8 per-core input maps."""
    in_maps = []
    hh = H // 2
    aux = aux_array()
    for core in range(N_CORES):
        b, half = divmod(core, 2)
        h0 = half * hh
        in_maps.append({
            "f_source": np.ascontiguousarray(
                f_source[b, :, h0:h0 + hh, :]).reshape(C, NPIX),
            "f_convert": np.ascontiguousarray(
                f_convert[b, :, h0:h0 + hh, :]).reshape(C, NPIX),
            "seg": np.ascontiguousarray(seg[b, h0:h0 + hh, :]).reshape(NPIX),
            "aux": aux,
        })
    return in_maps


def unpack_partial(p):
    """[2, 2, 128, 19] per-core partial -> (S, C) each [19, 256]."""
    s = np.concatenate([p[0, 0], p[0, 1]], axis=0).T
    c = np.concatenate([p[1, 0], p[1, 1]], axis=0).T
    return s, c


def epilogue(S, Csum):
    """Tiny triplet-loss tail on [19,256] class sums (float64 host math)."""
    n = float(B * H * W)
    cs = S.astype(np.float64) / n
    cc = Csum.astype(np.float64) / n
    cs = cs / np.maximum(np.linalg.norm(cs, axis=1, keepdims=True), EPS_NORM)
    cc = cc / np.maximum(np.linalg.norm(cc, axis=1, keepdims=True), EPS_NORM)
    D = np.linalg.norm(cs[:, None, :] - cc[None, :, :] + EPS_TRIP, axis=2)
    d_ap = np.diag(D)
    terms = np.maximum(d_ap[:, None] - D + MARGIN, 0.0)
    mask = 1.0 - np.eye(N_CLASS)
    loss = (terms * mask).sum() / (N_CLASS * (N_CLASS - 1))
    return np.float32(loss)


def kernel(f_source, f_convert, seg):
    if "nc" not in _NC_CACHE:
        _NC_CACHE["nc"] = build_nc()
    nc = _NC_CACHE["nc"]
    in_maps = shard_inputs(f_source, f_convert, seg)
    res = run_bass_kernel_spmd(nc, in_maps, core_ids=list(range(N_CORES)))
    S = np.zeros((N_CLASS, C), dtype=np.float64)
    Csum = np.zeros((N_CLASS, C), dtype=np.float64)
    for r in res.results:
        s, c = unpack_partial(r["out"].astype(np.float64))
        S += s
        Csum += c
    return epilogue(S, Csum)


if __name__ == "__main__":
    rng = np.random.default_rng(0)
    fs = rng.standard_normal((B, C, H, W), dtype=np.float32)
    fc = rng.standard_normal((B, C, H, W), dtype=np.float32)
    sg = rng.integers(0, N_CLASS, size=(B, H, W), dtype=np.int32)
    print(kernel(fs, fc, sg))


# revision 34
# speedup vs baseline: 1.1392x; 1.1392x over previous
"""Class-align loss (segment_reduce) Trainium2 kernel.

Full inputs: f_source [4,256,128,128] f32, f_convert [4,256,128,128] f32,
seg [4,128,128] int32 (values in [0,19)). Output: scalar f32 triplet loss.

Strategy (data-parallel over batch*h-half, 8 shards):
  - Each core processes a [256, 8192] shard of each feature tensor
    (1 batch x 64 h-rows x 128 w). Staging DMAs cast fp32 -> bf16 in
    flight (SWDGE); accumulation stays fp32 in PSUM.
  - Pixels are processed in batches of 4 groups x 128 pixels: PE
    transposes eight [128c,128p] bf16 blocks into one full-bank PSUM
    tile, one DVE copy moves it to SBUF; ACT computes per-pixel
    sum-of-squares (Square + accum) per group; one batched sqrt (ACT)
    + reciprocal (DVE) per batch gives r = 1/||x||.
  - Per-pixel normalization is folded into the one-hot class weights
    w[p,k] = (seg[p]==k) * r[p]; PE accumulates transposed class sums
    accT[c_half,k] += xT_half.T @ w (data is the 128-col stationary
    operand -> fast weight load; w is the 19-col moving operand).
  - Each core writes its partial [2,2,128,19] sums; the host sums the
    8 partials and computes the tiny (19-class) normalize +
    triplet-loss epilogue in float64.

The walrus build used here encodes at most ONE sync wait per
instruction. Everything below is arranged so no instruction ever needs
two: staging tiles are dedicated (wait-free DMAs), absorber transposes
take the staging-DMA waits on PE, sync=False ordering edges keep the PE
stream near program order so PSUM-slot WAR waits are subsumed by the
vector clock, and the kernel-tail drain is split across sequencer NOPs.
"""

import sys

import numpy as np

if "/opt/trn_rl_repo" not in sys.path:
    sys.path.insert(0, "/opt/trn_rl_repo")

import concourse.bass as bass
import concourse.mybir as mybir
import concourse.tile as tile
from concourse.bass_utils import run_bass_kernel_spmd
from concourse.tile import add_dep_helper
from concourse.vector_clock import ScopedClock


def _split_drain_and_barrier(self, tick_clock, wait_clock):
    """Tile's kernel-tail drain carries one wait per semaphore the kernel
    ever used; split the excess onto dedicated sequencer NOPs (the 1-wait
    walrus encoding limit)."""
    nc = self.nc
    drain_inst = nc.sync.drain()
    wait_clock.add_sem_waits(
        drain_inst.ins, ScopedClock({None: tick_clock.global_clock})
    )
    si = drain_inst.ins.sync_info
    if si is not None and len(si.on_wait) > 1:
        waits = list(si.on_wait)
        upds = list(si.on_update)
        drain_inst.ins.sync_info = mybir.SyncInfo(
            on_wait=waits[:1], on_update=upds)
        for k in range(1, len(waits)):
            nop = nc.sync.nop(nofuse=True, hint=f"drain_wait_{k}")
            nop.ins.sync_info = mybir.SyncInfo(
                on_wait=[waits[k]], on_update=[])
    nc.all_engine_barrier()
    assert self.sems is not None
    popped = nc._tile_sem_poison_stack.pop()
    assert popped is self._sem_poison
    nc.clear_and_free_semaphores(list(self.sems.allocated().values()))
    nc.all_engine_barrier()


tile.TileContext._drain_and_barrier = _split_drain_and_barrier

# Problem constants (hardcoded; kernel.py must be self-contained).
B, C, H, W = 4, 256, 128, 128
N_CLASS = 19
N_CORES = 8
EPS_NORM = 1e-12
EPS_TRIP = 1e-6
MARGIN = 0.2

P = 128                      # SBUF partitions / pixel-group size
NPIX = B * H * W // N_CORES  # 8192 pixels per core
NG = NPIX // P               # 64 pixel groups per core
GPB = 4                      # pixel groups per batch (one PSUM bank)
NB = NG // GPB               # 16 batches per tensor
CHUNK_PIX = 4096             # pixels staged per DMA
NCHUNK = NPIX // CHUNK_PIX   # 4
BPC = CHUNK_PIX // (P * GPB) # 4 batches per chunk

_NC_CACHE = {}


def build_nc():
    f32 = mybir.dt.float32
    bf16 = mybir.dt.bfloat16
    i32 = mybir.dt.int32
    nc = bass.Bass()

    fs_dram = nc.declare_dram_parameter("f_source", [C, NPIX], f32, isOutput=False)
    aux_dram = nc.declare_dram_parameter("aux", [P, P + N_CLASS], f32,
                                         isOutput=False)
    fc_dram = nc.declare_dram_parameter("f_convert", [C, NPIX], f32, isOutput=False)
    seg_dram = nc.declare_dram_parameter("seg", [NPIX], i32, isOutput=False)
    out_dram = nc.declare_dram_parameter("out", [2, 2, P, N_CLASS], f32,
                                         isOutput=True)

    with tile.TileContext(nc) as tc:
        with (
            tc.tile_pool(name="const", bufs=1) as const_pool,
            tc.tile_pool(name="stage", bufs=1) as stage_pool,
            tc.tile_pool(name="work", bufs=4) as work_pool,
            tc.tile_pool(name="wpool", bufs=16) as w_pool,
            tc.tile_pool(name="psum_t", bufs=3, space="PSUM") as psum_t_pool,
            tc.tile_pool(name="psum_abs", bufs=1, space="PSUM") as psum_abs_pool,
            tc.tile_pool(name="psum_acc", bufs=1, space="PSUM") as psum_acc_pool,
        ):
            # identity + iota row arrive via DMA (the "aux" input): building
            # them with gpsimd would add the Pool semaphore to every
            # wait-budget discussion.
            aux_sb = const_pool.tile([P, P + N_CLASS], f32, tag="aux")
            nc.gpsimd.dma_start(out=aux_sb[:], in_=aux_dram[:])
            iota19 = aux_sb[:, P:P + N_CLASS]
            ident_bf = const_pool.tile([P, P], bf16, tag="ident_bf")
            nc.vector.tensor_copy(ident_bf[:], aux_sb[:, 0:P])
            identity = ident_bf[:]

            # seg wanted as [pixel-within-group (partition), group (free)].
            # A strided gather DMA would cost 8192 descriptors (~48us of Q7
            # descriptor generation), so load contiguously and PE-transpose.
            seg_i = const_pool.tile([NG, P], i32, tag="seg_i")
            nc.gpsimd.dma_start(
                out=seg_i[:], in_=seg_dram[:].rearrange("(g p) -> g p", p=P))
            seg_f = const_pool.tile([NG, P], f32, tag="seg_f")
            nc.vector.tensor_copy(seg_f[:], seg_i[:])
            ident_sm = const_pool.tile([NG, NG], f32, tag="ident_sm")
            nc.vector.tensor_copy(ident_sm[:], aux_sb[:NG, :NG])
            seg_ps = psum_t_pool.tile([P, NG], f32, tag="pt", name="seg_ps",
                                      padded_shape=[P, 512])
            nc.tensor.transpose(seg_ps[:], seg_f[:], ident_sm[:])
            seg_sb = const_pool.tile([P, NG], f32, tag="seg")
            nc.vector.tensor_copy(seg_sb[:], seg_ps[:])

            # Dummy DVE read of iota19: pre-syncs DVE against the aux DMA so
            # the first w-generation op carries a single wait.
            iota_warm = const_pool.tile([P, N_CLASS], f32, tag="iota_warm")
            nc.vector.tensor_copy(iota_warm[:], iota19)

            # Warm-up transpose: pre-syncs PE against ident_bf (DVE).
            warm = psum_t_pool.tile([P, P], bf16, tag="pt", name="warm",
                                    padded_shape=[P, 1024])
            nc.tensor.transpose(warm[:, 0:P], identity, identity)

            # Transposed fp32 accumulators: accT[tensor][half] = [c_half, k].
            accs = {
                (t, h): psum_acc_pool.tile([P, N_CLASS], f32,
                                           tag=f"acc_{t}{h}", name=f"acc_{t}{h}")
                for t in ("s", "c") for h in (0, 1)
            }
            drams = {"s": fs_dram, "c": fc_dram}

            # Dedicated bank for the DMA-wait absorber transposes (never
            # read; lo/hi slices are byte-disjoint).
            absorb = psum_abs_pool.tile([P, 8 * P], bf16, tag="absorb",
                                        name="absorb", padded_shape=[P, 1024])

            mm_all = []

            def order_after_mm(inst, back=24):
                if len(mm_all) >= back:
                    add_dep_helper(inst.ins, mm_all[-back].ins, sync=False,
                                   reason="keep PE stream near program order")

            for ci in range(NCHUNK):
                for t in ("s", "c"):
                    # Dedicated staging tiles per (chunk, tensor, half): the
                    # DMAs carry zero waits. SWDGE casts fp32->bf16 in flight.
                    lo = stage_pool.tile([P, CHUNK_PIX], bf16,
                                         tag=f"{t}_lo_{ci}", name=f"{t}_lo_{ci}")
                    hi = stage_pool.tile([P, CHUNK_PIX], bf16,
                                         tag=f"{t}_hi_{ci}", name=f"{t}_hi_{ci}")
                    pix0 = ci * CHUNK_PIX
                    nc.gpsimd.dma_start(
                        out=lo[:], in_=drams[t][0:P, pix0:pix0 + CHUNK_PIX])
                    nc.gpsimd.dma_start(
                        out=hi[:], in_=drams[t][P:C, pix0:pix0 + CHUNK_PIX])
                    ab1 = nc.tensor.transpose(absorb[:, 0:P], lo[:, 0:P],
                                              identity)
                    ab2 = nc.tensor.transpose(absorb[:, P:2 * P], hi[:, 0:P],
                                              identity)
                    order_after_mm(ab1)
                    order_after_mm(ab2)
                    for bi in range(BPC):
                        batch = ci * BPC + bi            # global batch index
                        # One full PSUM bank holds 4 groups x [128p, 256c].
                        psumT = psum_t_pool.tile([P, GPB * C], bf16, tag="pt",
                                                 padded_shape=[P, 1024])
                        tps = []
                        for g in range(GPB):
                            px = (bi * GPB + g) * P
                            t1 = nc.tensor.transpose(
                                psumT[:, g * C:g * C + P],
                                lo[:, px:px + P], identity)
                            t2 = nc.tensor.transpose(
                                psumT[:, g * C + P:(g + 1) * C],
                                hi[:, px:px + P], identity)
                            tps.extend((t1, t2))
                        for tp in tps:
                            order_after_mm(tp)
                        xT = work_pool.tile([P, GPB * C], bf16, tag="xT",
                                              bufs=6)
                        nc.vector.tensor_copy(xT[:], psumT[:])

                        ss = work_pool.tile([P, GPB], f32, tag="ss", bufs=32)
                        if batch % 2 == 0:
                            # Even batches: per-group ACT Square+accum.
                            for g in range(GPB):
                                sq = work_pool.tile([P, C], bf16, tag="sq",
                                                    bufs=8)
                                nc.scalar.activation(
                                    sq[:], xT[:, g * C:(g + 1) * C],
                                    mybir.ActivationFunctionType.Square,
                                    accum_out=ss[:, g:g + 1])
                        else:
                            # Odd batches: one batched ACT square + DVE reduce.
                            sq4 = work_pool.tile([P, GPB * C], bf16, tag="sq4",
                                                 bufs=8)
                            nc.scalar.activation(
                                sq4[:], xT[:],
                                mybir.ActivationFunctionType.Square)
                            nc.vector.tensor_reduce(
                                out=ss[:],
                                in_=sq4[:].rearrange("p (g c) -> p g c", g=GPB),
                                axis=mybir.AxisListType.X,
                                op=mybir.AluOpType.add)
                        nrm = work_pool.tile([P, GPB], f32, tag="nrm", bufs=32)
                        nc.scalar.sqrt(nrm[:], ss[:])
                        r = work_pool.tile([P, GPB], f32, tag="r", bufs=32)
                        nc.vector.reciprocal(r[:], nrm[:])

                        for g in range(GPB):
                            G = batch * GPB + g
                            w = w_pool.tile([P, N_CLASS], bf16, tag="w")
                            nc.vector.tensor_scalar(
                                out=w[:], in0=iota19,
                                scalar1=seg_sb[:, G:G + 1],
                                scalar2=r[:, g:g + 1],
                                op0=mybir.AluOpType.is_equal,
                                op1=mybir.AluOpType.mult)
                            for h in (0, 1):
                                mm = nc.tensor.matmul(
                                    accs[(t, h)][:],
                                    lhsT=xT[:, g * C + h * P:g * C + (h + 1) * P],
                                    rhs=w[:],
                                    start=(G == 0), stop=(G == NG - 1))
                                mm_all.append(mm)

            out_sb = work_pool.tile([P, 4 * N_CLASS], f32, tag="out_sb")
            for j, (t, h) in enumerate(((("s", 0)), ("s", 1), ("c", 0), ("c", 1))):
                nc.vector.tensor_copy(
                    out_sb[:, j * N_CLASS:(j + 1) * N_CLASS], accs[(t, h)][:])
            for j, (ti, h) in enumerate(((0, 0), (0, 1), (1, 0), (1, 1))):
                # HWDGE lanes are otherwise unused -> each DMA carries only
                # its DVE wait.
                nc.sync.dma_start(
                    out=out_dram[ti, h],
                    in_=out_sb[:, j * N_CLASS:(j + 1) * N_CLASS])

    return nc


def aux_array():
    ident = np.eye(P, dtype=np.float32)
    iota = np.tile(np.arange(N_CLASS, dtype=np.float32), (P, 1))
    return np.ascontiguousarray(np.concatenate([ident, iota], axis=1))


def shard_inputs(f_source, f_convert, seg):
    """Split by (batch, h-half) into 8 per-core input maps."""
    in_maps = []
    hh = H // 2
    aux = aux_array()
    for core in range(N_CORES):
        b, half = divmod(core, 2)
        h0 = half * hh
        in_maps.append({
            "f_source": np.ascontiguousarray(
                f_source[b, :, h0:h0 + hh, :]).reshape(C, NPIX),
            "f_convert": np.ascontiguousarray(
                f_convert[b, :, h0:h0 + hh, :]).reshape(C, NPIX),
            "seg": np.ascontiguousarray(seg[b, h0:h0 + hh, :]).reshape(NPIX),
            "aux": aux,
        })
    return in_maps


def unpack_partial(p):
    """[2, 2, 128, 19] per-core partial -> (S, C) each [19, 256]."""
    s = np.concatenate([p[0, 0], p[0, 1]], axis=0).T
    c = np.concatenate([p[1, 0], p[1, 1]], axis=0).T
    return s, c


def epilogue(S, Csum):
    """Tiny triplet-loss tail on [19,256] class sums (float64 host math)."""
    n = float(B * H * W)
    cs = S.astype(np.float64) / n
    cc = Csum.astype(np.float64) / n
    cs = cs / np.maximum(np.linalg.norm(cs, axis=1, keepdims=True), EPS_NORM)
    cc = cc / np.maximum(np.linalg.norm(cc, axis=1, keepdims=True), EPS_NORM)
    D = np.linalg.norm(cs[:, None, :] - cc[None, :, :] + EPS_TRIP, axis=2)
    d_ap = np.diag(D)
    terms = np.maximum(d_ap[:, None] - D + MARGIN, 0.0)
    mask = 1.0 - np.eye(N_CLASS)
    loss = (terms * mask).sum() / (N_CLASS * (N_CLASS - 1))
    return np.float32(loss)


def kernel(f_source, f_convert, seg):
    if "nc" not in _NC_CACHE:
        _NC_CACHE["nc"] = build_nc()
    nc = _NC_CACHE["nc"]
    in_maps = shard_inputs(f_source, f_convert, seg)
    res = run_bass_kernel_spmd(nc, in_maps, core_ids=list(range(N_CORES)))
    S = np.zeros((N_CLASS, C), dtype=np.float64)
    Csum = np.zeros((N_CLASS, C), dtype=np.float64)
    for r in res.results:
        s, c = unpack_partial(r["out"].astype(np.float64))
        S += s
        Csum += c
    return epilogue(S, Csum)


if __name__ == "__main__":
    rng = np.random.default_rng(0)
    fs = rng.standard_normal((B, C, H, W), dtype=np.float32)
    fc = rng.standard_normal((B, C, H, W), dtype=np.float32)
    sg = rng.integers(0, N_CLASS, size=(B, H, W), dtype=np.int32)
    print(kernel(fs, fc, sg))
